# revision 40
# baseline (speedup 1.0000x reference)
"""AdaptiveNeuromorphicNetwork Trainium2 kernel (8 NeuronCores, SPMD).

Sharding: output neurons H=2048 split 256/core (H-shard) -> the LIF scan,
spike-rate mean (over batch) and homeostatic threshold update are fully local
per core; zero collectives. input_spikes are replicated (each core streams all
of them through the TensorEngine against its weight column shard).

Per-core pipeline over time-chunks:
  DMA fp8 spikes (host-relayouted, contiguous) -> matmul W^T-stationary into
  PSUM -> evac PSUM->SBUF (scalar engine, descale) -> sequential LIF scan
  (vector engine, fused ops; threshold EMA chain on gpsimd) -> spikes
  accumulate in SBUF -> chunked DMA out.

Matmul precision (MATMUL_MODE="fp8x4", the default): W pre-scaled by 2^8 and
decomposed into 4 residual fp8-e4m3 planes (w*2^8 = p0+p1+p2+p3, each plane
the fp8 rounding of the remaining residual). All four planes run DoubleRow
(contracts 2 k-tiles per matmul at 0.5 cyc/row -> 0.25 cyc per k-tile*column,
4x cheaper than fp16), accumulating into one PSUM group at the shared 2^8
scale; descaled by 2^-8 in the scalar-engine evacuation. Spikes are exactly
0/1 in fp8 so a single fp8 spike DMA feeds all planes -- no fp16 spike copy,
no on-device casts. Effective weight error ~max(2^-16*|w|, 2^-18) -> output
rel err ~5e-3 (gate 2e-2). Legacy modes kept: "bf16fp8" (fp16 hi + fp8
DoubleRow residual, 1.25 cyc/ktile-col), "f32" (exact, 4 cyc), "bf16x2".

The LIF scan uses two custom fused DVE ops (registered at import):
  LIF_S: s = ((a_mem*v + i_syn) + negThr) >= 0, accum_out = sum_b(s)
  LIF_V: v' = P + s*negThr  (recomputes P, s internally)
run per h-tile per step (per-partition negThr), plus one stock
scalar_tensor_tensor for the i_syn EMA. The homeostatic threshold EMA is
algebraically folded to 5 tensor_tensor ops on gpsimd:
  R' = 0.99R + 0.01*lr*tgt + (-lr/6400)*rateSum ; negThr += R'
(R = lr*tgt - lr*fre; rateSum from the fused LIF_S accumulator).
"""
import numpy as np

import concourse.bass as bass
import concourse.tile as tile
from concourse import bacc, mybir
from concourse.bass_utils import run_bass_kernel_spmd

B, I, H, T = 64, 2048, 2048, 128
NCORES = 8
HL = H // NCORES            # 256 output neurons per core
KT = I // 128               # 16 contraction tiles
CHUNKS = [2, 4, 8, 8, 8, 16, 16, 16, 16, 16, 16, 2]   # per-chunk step counts
NCH = len(CHUNKS)
assert sum(CHUNKS) == T
DT = 0.001

MATMUL_MODE = "fp8x4"
NPLANES = 3         # fp8 residual planes (fp8x4 mode)
S_EXP = 8           # legacy; fp8x4 uses the adaptive W_SCALE below
W_SCALE_NUM = 238.0  # plane-0 peak target: scale = 238/max|w| (fp8 max 240)
ISYN_ON_GPSIMD = False  # i_syn stays on DVE; 4-slot ring lets it run ahead
TRACE = False
TRACE_KW = {}
REPEAT = 1          # execute the whole pipeline N times (timing builds only)
CAST_HI = True      # bf16fp8: DMA only fp8 spikes; build fp16 copies on-device
SPK_ONE_DMA = True  # one rearranged spike DMA per chunk (vs per-k-tile)
WARMUP_MM = 0     # dummy matmuls to ramp the PE p-state during DMA fill

_F32 = mybir.dt.float32
_ALU = mybir.AluOpType

# ---- custom fused DVE ops for the LIF step ----
import operator as _op

import concourse.dve_ops as _dve_ops
from concourse.dve_ops import DveOp as _DveOp
from concourse.dve_spec import (Spec as _Spec, Src0 as _Src0, Src1 as _Src1,
                                C0 as _C0, C1 as _C1, C2 as _C2, Zero as _Zero,
                                lower as _lower, _has_src1)
from concourse.dve_table_gen import dve_ver_for as _dve_ver_for
from concourse.dve_uop import DveOpSpec as _DveOpSpec


def _register_dve(name, spec):
    if name in _dve_ops._SUB_OPCODE_FOR_NAME:
        for o in _dve_ops.OPS:
            if o.name == name:
                return o
    ver = _dve_ver_for("TRN2")
    opcode = max(_dve_ops._SUB_OPCODE_FOR_NAME.values()) + 1
    assert opcode < 0x20
    sha = _DveOpSpec(name=name, opcode=opcode, uops=_lower(spec, ver=ver),
                     rd1_en=_has_src1(spec)).sha(ver)
    dop = _DveOp(name, spec, subdim=False, uops_sha={ver: sha})
    _dve_ops.OPS.append(dop)
    _dve_ops.CUSTOM_DVE_SPECS[name] = spec
    _dve_ops._SUB_OPCODE_FOR_NAME[name] = opcode
    return dop


def _lif_s_ref(in0, in1, s0, s1, imm2):
    P = in1.astype(np.float32) * s0 + in0
    s = (P + s1 >= 0).astype(np.float32)
    return s, s.reshape(s.shape[0], -1).sum(axis=-1, keepdims=True)


def _lif_v_ref(in0, in1, s0, s1, imm2):
    P = in1.astype(np.float32) * s0 + in0
    s = (P + s1 >= 0).astype(np.float32)
    return P + s * s1


# s = ((v*a_mem + isyn) + negThr) >= 0 ; accum = sum(s) over free dim
_P = _Src1 * _C0 + _Src0
LIF_S = _register_dve("LIF_S", _Spec(body=(_P + _C1) >= _Zero,
                                     accum=_op.add, reference=_lif_s_ref))


def _lif_sc_ref(in0, in1, s0, s1, imm2):
    P = in1.astype(np.float32) * s0 + in0
    s = (P + s1 >= 0).astype(np.float32) * imm2
    return s, s.reshape(s.shape[0], -1).sum(axis=-1, keepdims=True)


# s' = (((v*a_mem + isyn) + negThr) >= 0) * cc ; accum = sum(s') = cc*rate
# (output is cc-scaled spikes; host recovers spikes as (out != 0))
LIF_SC = _register_dve("LIF_SC",
                       _Spec(body=((_P + _C1) >= _Zero) * _C2,
                             accum=_op.add, reference=_lif_sc_ref))
# v' = P + ((P + negThr) >= 0) * negThr
LIF_V = _register_dve("LIF_V", _Spec(body=_P + ((_P + _C1) >= _Zero) * _C1,
                                     reference=_lif_v_ref))


def _mm_dtype():
    return {"f32": mybir.dt.float32, "f32r": mybir.dt.float32r,
            "bf16x2": mybir.dt.bfloat16,
            "bf16fp8": mybir.dt.float16,
            "fp8x4": mybir.dt.float8e4}[MATMUL_MODE]


LO_SCALE = 2.0 ** 14


def _col_blocks(n, blk=512):
    """Split n columns into PSUM-bank-sized (<=512 f32) blocks."""
    return [(c, min(c + blk, n)) for c in range(0, n, blk)]


def _build_fp8x4(a_mem, a_syn, lr, tgt, wscale):
    """All-fp8 multi-plane DoubleRow pipeline (see module docstring)."""
    nc = bacc.Bacc("TRN2", target_bir_lowering=False, debug=False,
                   num_devices=NCORES)
    f8 = mybir.dt.float8e4
    NP = NPLANES
    # weights: [i128, (plane, kp, ht, ko, h)] -> per-plane contiguous DMAs
    wgt8 = nc.dram_tensor("wgt8", [128, NP * KT * 2 * 128], f8,
                          kind="ExternalInput").ap()
    spk8 = nc.dram_tensor("spk8", [KT * 128, B * T], f8,
                          kind="ExternalInput").ap()
    nt0 = nc.dram_tensor("nt0", [128, 2], _F32, kind="ExternalInput").ap()
    odt = mybir.dt.bfloat16
    out = nc.dram_tensor("out", [128, T * 128], odt, kind="ExternalOutput").ap()

    a_mem, a_syn, lr, tgt = float(a_mem), float(a_syn), float(lr), float(tgt)
    c_ema = float(np.float32(-lr / 6400.0))
    k1 = float(np.float32(0.01 * lr * tgt))
    r0 = float(np.float32(lr * tgt))
    descale = 1.0 / float(wscale)
    PL = KT * 2 * 128           # per-plane weight columns

    with tile.TileContext(nc) as tc:
        with tc.tile_pool(name="wpool", bufs=1) as wpool, \
             tc.tile_pool(name="state", bufs=1) as state, \
             tc.tile_pool(name="spkp", bufs=3) as spkp, \
             tc.tile_pool(name="psum", bufs=2, space="PSUM") as psum, \
             tc.tile_pool(name="wev", bufs=6) as wev, \
             tc.tile_pool(name="ipool", bufs=2) as ipool, \
             tc.tile_pool(name="accp", bufs=3) as accp, \
             tc.tile_pool(name="tmp", bufs=3) as tmp:

            # ---- persistent tiles ----
            # DMA order: weight plane 0, chunk-0 spikes, then later planes --
            # per-plane weight tiles so plane-0 matmuls only depend on the
            # plane-0 DMA.
            # All startup DMAs on the sync queue, in the exact order the
            # (serial) DMA engines should service them: all weight planes
            # (one DMA; the last plane gates chunk 0's PSUM stop anyway),
            # then chunk-0 spikes. The chunk loop's c>=1 spike DMAs queue
            # behind on the same queue.
            wsb8 = wpool.tile([128, NP * PL], f8, tag="wsb8")
            wsb8p = [wsb8[:, p * PL:(p + 1) * PL] for p in range(NP)]
            nc.sync.dma_start(wsb8[:], wgt8[:])
            spk_c0 = spkp.tile([128, KT * B * CHUNKS[0]], f8, tag="spk8",
                               name="spk8_c0")
            src0 = spk8[:, 0:B * CHUNKS[0]].rearrange("(k p) n -> p k n", k=KT)
            nc.sync.dma_start(
                spk_c0[:].rearrange("p (k n) -> p k n", k=KT), src0)
            # segment masks for the i_syn tensor_tensor_scan: a_syn
            # everywhere, 0 at each (h,b) segment's t=0 column (gpsimd:
            # keeps the DVE free for the scan)
            masks = {}
            for TCv in sorted(set(CHUNKS)):
                mk = state.tile([128, 128 * TCv], _F32, tag=f"mask{TCv}",
                                name=f"mask{TCv}")
                nc.gpsimd.memset(mk[:], a_syn)
                m3 = mk[:].rearrange("p (m t) -> p m t", t=TCv)
                nc.gpsimd.memset(m3[:, :, 0:1], 0.0)
                masks[TCv] = mk
            # negThr double-buffer: step t reads nTs[t%2], the critical
            # update writes nTs[(t+1)%2]
            nTs = [state.tile([128, 2], _F32, tag=f"nT{i}", name=f"nT{i}")
                   for i in range(2)]
            nc.scalar.dma_start(nTs[0][:], nt0[:])
            # T1 = 0.99*R + k1 and NT1 = nT + T1, maintained off the
            # rs->nT critical path (gpsimd)
            T1st = state.tile([128, 2], _F32, tag="T1st")
            nc.gpsimd.memset(T1st[:], float(np.float32(0.99 * r0 + k1)))
            NT1st = state.tile([128, 2], _F32, tag="NT1st")
            nc.gpsimd.tensor_tensor(NT1st[:], nTs[0][:], T1st[:], op=_ALU.add)
            K1t = state.tile([128, 2], _F32, tag="K1t")
            nc.vector.memset(K1t[:], k1)
            C99t = state.tile([128, 2], _F32, tag="C99t")
            nc.gpsimd.memset(C99t[:], 0.99)
            vst = [state.tile([128, 128], _F32, tag=f"v{i}", name=f"v{i}")
                   for i in range(2)]
            nc.vector.memset(vst[0][:], 0.0)

            # PE p-state warmup: dummy matmuls (zero operands, results
            # discarded -- chunk 0's real start=True group resets the PSUM)
            # keep the PE busy while the startup DMAs stream, so the real
            # matmuls run at full clock from the first chunk.
            ps_c0 = [psum.tile([128, B * CHUNKS[0]], _F32, tag=f"ps{ht}",
                               name=f"ps_c0_{ht}") for ht in range(2)]
            warm = state.tile([128, 128], f8, tag="warm")
            nc.vector.memset(warm[:], 0.0)
            for _w in range(WARMUP_MM):
                nc.tensor.matmul(ps_c0[0][:, 0:min(128, B * CHUNKS[0])],
                                 warm[:], warm[:], start=True, stop=True,
                                 skip_group_check=True)

            for _rep in range(REPEAT):
                t0 = 0
                i_prev, TCp = None, 0     # previous chunk's i_syn tile
                for c in range(NCH):
                    TC = CHUNKS[c]
                    BTC = B * TC
                    cols0 = B * t0      # column offset into per-k row space
                    # ---- fp8 spike DMA for this chunk ----
                    if c == 0 and REPEAT == 1:
                        spk8_t = spk_c0
                    else:
                        spk8_t = spkp.tile([128, KT * BTC], f8, tag="spk8",
                                           name=f"spk8_c{c}")
                        src = spk8[:, cols0:cols0 + BTC].rearrange(
                            "(k p) n -> p k n", k=KT)
                        dst = spk8_t[:].rearrange("p (k n) -> p k n", k=KT)
                        nc.sync.dma_start(dst, src)
                    # ---- multi-plane DoubleRow matmul, shared PSUM group ----
                    if c == 0 and REPEAT == 1:
                        ps = ps_c0
                    else:
                        ps = [psum.tile([128, BTC], _F32, tag=f"ps{ht}",
                                        name=f"ps{c}_{ht}") for ht in range(2)]
                    blocks = _col_blocks(BTC)
                    for p in range(NP):
                        for kp in range(KT // 2):
                            for ht in range(2):
                                l8 = wsb8p[p][:, ((kp * 2 + ht) * 2) * 128:
                                              ((kp * 2 + ht) * 2 + 2) * 128
                                              ].rearrange("p (ko h) -> p ko h",
                                                          ko=2)
                                r8 = spk8_t[:, (2 * kp) * BTC:
                                            (2 * kp + 2) * BTC].rearrange(
                                    "p (ko n) -> p ko n", ko=2)
                                for c0, c1 in blocks:
                                    nc.tensor.matmul(
                                        ps[ht][:, c0:c1],
                                        l8, r8[:, :, c0:c1],
                                        start=(p == 0 and kp == 0),
                                        stop=(p == NP - 1
                                              and kp == KT // 2 - 1),
                                        perf_mode=mybir.MatmulPerfMode.DoubleRow)
                    # ---- evacuate PSUM -> SBUF (scalar engine, descale) ----
                    # wt_ev layout: [p, (h, b, t)]
                    wt_ev = wev.tile([128, 2 * BTC], _F32, tag="wt_ev")
                    with tc.high_priority():
                        for ht in range(2):
                            nc.scalar.activation(
                                wt_ev[:, ht * BTC:(ht + 1) * BTC],
                                ps[ht][:],
                                mybir.ActivationFunctionType.Copy,
                                bias=0.0, scale=descale)

                    # ---- i_syn for the whole chunk: one segmented
                    # tensor_tensor_scan (state = mask*state + w; the mask's
                    # 0 at each segment's t=0 column resets the recurrence,
                    # and the first w column is pre-fixed to a_syn*carry + w0)
                    wv = wt_ev[:].rearrange("p (m t) -> p m t", t=TC)
                    if i_prev is not None:
                        pv = i_prev[:].rearrange("p (m t) -> p m t", t=TCp)
                        nc.vector.scalar_tensor_tensor(
                            wv[:, :, 0:1], pv[:, :, TCp - 1:TCp], a_syn,
                            wv[:, :, 0:1], op0=_ALU.mult, op1=_ALU.add)
                    i_all = ipool.tile([128, 2 * BTC], _F32, tag="i_all",
                                       name=f"i_all_c{c}")
                    nc.vector.tensor_tensor_scan(
                        i_all[:], masks[TC][:], wt_ev[:], 0.0,
                        op0=_ALU.mult, op1=_ALU.add)
                    i4 = i_all[:].rearrange("p (h b t) -> p h b t", h=2, b=B)
                    i_prev, TCp = i_all, TC

                    # ---- LIF scan over this chunk ----
                    # nT' = NT1 + cc*rs is the only op on the step-to-step
                    # critical path (one DVE TT right after the LIF_SC pair).
                    # T1' = 0.99*(T1 + cc*rs) + k1 and NT1' = nT' + T1' are
                    # maintained on gpsimd, off the critical path.
                    acc = accp.tile([128, TC * 128], odt, tag="acc")
                    for tl in range(TC):
                        t = t0 + tl
                        last = (t == T - 1)
                        vold, vnew = vst[t % 2], vst[(t + 1) % 2]
                        nTo, nTn = nTs[t % 2], nTs[(t + 1) % 2]
                        rs = tmp.tile([128, 2], _F32, tag="rs")
                        for ht in range(2):
                            s_out = acc[:, tl * 128 + ht * B:
                                        tl * 128 + (ht + 1) * B]
                            # s' = cc*(((a_mem*v + i) + nT) >= 0) ;
                            # rs = sum_b s' = cc*rate
                            nc.vector._custom_dve(
                                LIF_SC, out=s_out,
                                in0=i4[:, ht, :, tl],
                                in1=vold[:, ht * B:(ht + 1) * B], s0=a_mem,
                                s1=nTo[:, ht:ht + 1], imm2=c_ema,
                                accum_out=rs[:, ht:ht + 1])
                        if not last:
                            # critical: nT' = NT1 + cc*rs  (DVE, in-queue)
                            nc.vector.tensor_tensor(nTn[:], NT1st[:], rs[:],
                                                    op=_ALU.add)
                            for ht in range(2):
                                sl = slice(ht * B, (ht + 1) * B)
                                # v' = P + s*nT (recomputes s from old nT)
                                nc.vector._custom_dve(
                                    LIF_V, out=vnew[:, sl],
                                    in0=i4[:, ht, :, tl],
                                    in1=vold[:, sl], s0=a_mem,
                                    s1=nTo[:, ht:ht + 1])
                            # off-path threshold state (gpsimd):
                            # u = T1 + cc*rs ; u2 = 0.99*u ; T1' = u2 + k1 ;
                            # NT1' = nT' + T1'
                            u = tmp.tile([128, 2], _F32, tag="u")
                            nc.gpsimd.tensor_tensor(u[:], T1st[:], rs[:],
                                                    op=_ALU.add)
                            nc.gpsimd.tensor_tensor(u[:], u[:], C99t[:],
                                                    op=_ALU.mult)
                            nc.gpsimd.tensor_tensor(T1st[:], u[:], K1t[:],
                                                    op=_ALU.add)
                            nc.gpsimd.tensor_tensor(NT1st[:], nTn[:], T1st[:],
                                                    op=_ALU.add)
                    # ---- chunk output ----
                    nc.sync.dma_start(out[:, t0 * 128:(t0 + TC) * 128], acc[:])
                    t0 += TC
    nc.compile()
    return nc


def _build(a_mem, a_syn, lr, tgt, wscale=None):
    """Build + compile the per-core Bass graph (same graph on all 8 cores)."""
    if MATMUL_MODE == "fp8x4":
        return _build_fp8x4(a_mem, a_syn, lr, tgt, wscale)
    nc = bacc.Bacc("TRN2", target_bir_lowering=False, debug=False,
                   num_devices=NCORES)
    mdt = _mm_dtype()
    npass = 2 if MATMUL_MODE == "bf16x2" else 1
    fp8lo = MATMUL_MODE == "bf16fp8"
    # spikes: flat, per chunk c: KT blocks of [128, B*tc] contiguous
    if not (fp8lo and CAST_HI):
        spk = nc.dram_tensor("spk", [KT * 128, B * T], mdt,
                             kind="ExternalInput").ap()
    # weights: [i128, (pass,k,ht,h)] -> one contiguous DMA
    wgt = nc.dram_tensor("wgt", [128, npass * KT * 2 * 128], mdt,
                         kind="ExternalInput").ap()
    if fp8lo:
        wgt8 = nc.dram_tensor("wgt8", [128, KT * 2 * 128], mybir.dt.float8e4,
                              kind="ExternalInput").ap()
        spk8 = nc.dram_tensor("spk8", [KT * 128, B * T], mybir.dt.float8e4,
                              kind="ExternalInput").ap()
    nt0 = nc.dram_tensor("nt0", [128, 2], _F32, kind="ExternalInput").ap()
    odt = mybir.dt.bfloat16 if fp8lo else _F32
    out = nc.dram_tensor("out", [128, T * 128], odt, kind="ExternalOutput").ap()

    a_mem, a_syn, lr, tgt = float(a_mem), float(a_syn), float(lr), float(tgt)
    c_ema = float(np.float32(-lr / 6400.0))
    k1 = float(np.float32(0.01 * lr * tgt))
    r0 = float(np.float32(lr * tgt))

    with tile.TileContext(nc) as tc:
        with tc.tile_pool(name="wpool", bufs=1) as wpool, \
             tc.tile_pool(name="state", bufs=1) as state, \
             tc.tile_pool(name="spkp", bufs=2) as spkp, \
             tc.tile_pool(name="psum", bufs=2, space="PSUM") as psum, \
             tc.tile_pool(name="wev", bufs=3) as wev, \
             tc.tile_pool(name="accp", bufs=3) as accp, \
             tc.tile_pool(name="tmp", bufs=3) as tmp:

            # ---- persistent tiles ----
            wsb = wpool.tile([128, npass * KT * 2 * 128], mdt, tag="wsb")
            nc.scalar.dma_start(wsb[:], wgt[:])
            if fp8lo:
                wsb8 = wpool.tile([128, KT * 2 * 128], mybir.dt.float8e4,
                                  tag="wsb8")
                nc.scalar.dma_start(wsb8[:], wgt8[:])
            nT = state.tile([128, 2], _F32, tag="nT")
            nc.sync.dma_start(nT[:], nt0[:])
            Rst = state.tile([128, 2], _F32, tag="Rst")
            nc.vector.memset(Rst[:], r0)
            K1t = state.tile([128, 2], _F32, tag="K1t")
            nc.vector.memset(K1t[:], k1)
            C99t = state.tile([128, 2], _F32, tag="C99t")
            nc.gpsimd.memset(C99t[:], 0.99)
            Cct = state.tile([128, 2], _F32, tag="Cct")
            nc.gpsimd.memset(Cct[:], c_ema)
            Casyn = state.tile([128, 128], _F32, tag="Casyn")
            nc.gpsimd.memset(Casyn[:], a_syn)
            vst = [state.tile([128, 128], _F32, tag=f"v{i}", name=f"v{i}") for i in range(2)]
            ist = [state.tile([128, 128], _F32, tag=f"i{i}", name=f"isyn{i}") for i in range(4)]
            nc.vector.memset(vst[0][:], 0.0)
            nc.vector.memset(ist[0][:], 0.0)

            for _rep in range(REPEAT):
                t0 = 0
                for c in range(NCH):
                    TC = CHUNKS[c]
                    cols0 = B * t0          # column offset into per-k row space
                    # ---- matmul chunk: weighted[h, (b,t)] for t in chunk ----
                    spk_t = spkp.tile([128, KT * B * TC], mdt, tag="spk",
                                      name=f"spk_c{c}")
                    if fp8lo and CAST_HI:
                        spk8_t = spkp.tile([128, KT * B * TC],
                                           mybir.dt.float8e4,
                                           tag="spk8", name=f"spk8_c{c}")
                        for k in range(KT):
                            sl8 = slice(k * B * TC, (k + 1) * B * TC)
                            nc.sync.dma_start(
                                spk8_t[:, sl8],
                                spk8[k * 128:(k + 1) * 128,
                                     cols0:cols0 + B * TC])
                            if k % 16 < 9:
                                nc.scalar.copy(spk_t[:, sl8], spk8_t[:, sl8])
                            else:
                                nc.vector.tensor_copy(spk_t[:, sl8],
                                                      spk8_t[:, sl8])
                    else:
                        for k in range(KT):
                            nc.sync.dma_start(
                                spk_t[:, k * B * TC:(k + 1) * B * TC],
                                spk[k * 128:(k + 1) * 128,
                                    cols0:cols0 + B * TC])
                        if fp8lo:
                            spk8_t = spkp.tile([128, KT * B * TC],
                                               mybir.dt.float8e4,
                                               tag="spk8", name=f"spk8_c{c}")
                            for k in range(KT):
                                sl8 = slice(k * B * TC, (k + 1) * B * TC)
                                if k % 2 == 0:
                                    nc.sync.dma_start(
                                        spk8_t[:, sl8],
                                        spk8[k * 128:(k + 1) * 128,
                                             cols0:cols0 + B * TC])
                                elif k % 4 == 1:
                                    nc.scalar.copy(spk8_t[:, sl8],
                                                   spk_t[:, sl8])
                                else:
                                    nc.vector.tensor_copy(spk8_t[:, sl8],
                                                          spk_t[:, sl8])
                    ps = [psum.tile([128, B * TC], _F32, tag=f"ps{ht}", name=f"ps{c}_{ht}")
                          for ht in range(2)]
                    nmm = KT * npass + (KT // 2 if fp8lo else 0)
                    nblk = B * TC // 512
                    for k in range(KT):
                        for p in range(npass):
                            mi = k * npass + p
                            for ht in range(2):
                                lhsT = wsb[:, ((p * KT + k) * 2 + ht) * 128:
                                           ((p * KT + k) * 2 + ht + 1) * 128]
                                for blk in range(nblk):
                                    nc.tensor.matmul(
                                        ps[ht][:, blk * 512:(blk + 1) * 512],
                                        lhsT,
                                        spk_t[:, k * B * TC + blk * 512:
                                              k * B * TC + (blk + 1) * 512],
                                        start=(mi == 0),
                                        stop=(mi == nmm - 1 and not fp8lo))
                    if fp8lo:
                        # lo pass: fp8 DoubleRow over k-pairs, same psum group
                        for kp in range(KT // 2):
                            for ht in range(2):
                                l8 = wsb8[:, ((kp * 2 + ht) * 2) * 128:
                                          ((kp * 2 + ht) * 2 + 2) * 128].rearrange(
                                    "p (ko h) -> p ko h", ko=2)
                                r8 = spk8_t[:, (2 * kp) * B * TC:
                                            (2 * kp + 2) * B * TC].rearrange(
                                    "p (ko n) -> p ko n", ko=2)
                                for blk in range(nblk):
                                    nc.tensor.matmul(
                                        ps[ht][:, blk * 512:(blk + 1) * 512],
                                        l8, r8[:, :, blk * 512:(blk + 1) * 512],
                                        start=False, stop=(kp == KT // 2 - 1),
                                        perf_mode=mybir.MatmulPerfMode.DoubleRow)
                    # ---- evacuate PSUM -> SBUF (scalar engine, descale) ----
                    wt_ev = wev.tile([128, 2 * B * TC], _F32, tag="wt_ev")
                    with tc.high_priority():
                        for ht in range(2):
                            if fp8lo:
                                nc.scalar.activation(
                                    wt_ev[:, ht * B * TC:(ht + 1) * B * TC],
                                    ps[ht][:],
                                    mybir.ActivationFunctionType.Copy,
                                    bias=0.0, scale=1.0 / LO_SCALE)
                            else:
                                nc.scalar.copy(
                                    wt_ev[:, ht * B * TC:(ht + 1) * B * TC],
                                    ps[ht][:])
                    w3 = wt_ev[:].rearrange("p (h b t) -> p h b t", h=2, b=B)

                    # ---- LIF scan over this chunk ----
                    acc = accp.tile([128, TC * 128], odt, tag="acc")
                    for tl in range(TC):
                        t = t0 + tl
                        iold, inew = ist[t % 4], ist[(t + 1) % 4]
                        vold, vnew = vst[t % 2], vst[(t + 1) % 2]
                        i3o = iold[:].rearrange("p (h b) -> p h b", h=2)
                        i3n = inew[:].rearrange("p (h b) -> p h b", h=2)
                        # i_syn = a_syn*i_syn + weighted[t]
                        if bool(ISYN_ON_GPSIMD):
                            nc.gpsimd.tensor_tensor(i3n, i3o,
                                                    Casyn[:].rearrange(
                                                        "p (h b) -> p h b", h=2),
                                                    op=_ALU.mult)
                            nc.gpsimd.tensor_tensor(i3n, i3n, w3[:, :, :, tl],
                                                    op=_ALU.add)
                        else:
                            nc.vector.scalar_tensor_tensor(
                                i3n, i3o, a_syn, w3[:, :, :, tl],
                                op0=_ALU.mult, op1=_ALU.add)
                        rs = tmp.tile([128, 2], _F32, tag="rs")
                        for ht in range(2):
                            sl = slice(ht * B, (ht + 1) * B)
                            s_out = acc[:, tl * 128 + ht * B: tl * 128 + (ht + 1) * B]
                            # s = ((a_mem*v + i) + nT) >= 0 ; rs = sum_b s  (vector)
                            nc.vector._custom_dve(
                                LIF_S, out=s_out, in0=inew[:, sl], in1=vold[:, sl],
                                s0=a_mem, s1=nT[:, ht:ht + 1],
                                accum_out=rs[:, ht:ht + 1])
                            # v' = P + s*nT                               (vector)
                            nc.vector._custom_dve(
                                LIF_V, out=vnew[:, sl], in0=inew[:, sl],
                                in1=vold[:, sl], s0=a_mem, s1=nT[:, ht:ht + 1])
                        # threshold EMA chain (gpsimd, TT-only ops)
                        # R' = 0.99*R + 0.01*lr*tgt + (-lr/6400)*rsum ; nT += R'
                        t1 = tmp.tile([128, 2], _F32, tag="t1")
                        nc.gpsimd.tensor_tensor(t1[:], Rst[:], C99t[:], op=_ALU.mult)
                        nc.gpsimd.tensor_tensor(t1[:], t1[:], K1t[:], op=_ALU.add)
                        t2 = tmp.tile([128, 2], _F32, tag="t2")
                        nc.gpsimd.tensor_tensor(t2[:], rs[:], Cct[:], op=_ALU.mult)
                        nc.gpsimd.tensor_tensor(Rst[:], t2[:], t1[:], op=_ALU.add)
                        nc.gpsimd.tensor_tensor(nT[:], nT[:], Rst[:], op=_ALU.add)
                    # ---- chunk output ----
                    nc.sync.dma_start(out[:, t0 * 128:(t0 + TC) * 128], acc[:])
                    t0 += TC
    nc.compile()
    return nc


_CACHE = {}


def _get_nc(a_mem, a_syn, lr, tgt, wscale=None):
    key = (MATMUL_MODE, ISYN_ON_GPSIMD, REPEAT, CAST_HI, NPLANES, S_EXP,
           SPK_ONE_DMA, tuple(CHUNKS), wscale,
           float(a_mem), float(a_syn), float(lr), float(tgt))
    if key not in _CACHE:
        _CACHE[key] = _build(a_mem, a_syn, lr, tgt, wscale)
    return _CACHE[key]


def kernel(input_spikes, weight, synaptic_strength, threshold,
           tau_mem, tau_syn, target_rate, homeostatic_lr):
    spikes = np.asarray(input_spikes, dtype=np.float32)
    w_eff = (np.asarray(weight, dtype=np.float32)
             * np.asarray(synaptic_strength, dtype=np.float32))
    thr = np.asarray(threshold, dtype=np.float32)
    tau_m = np.float32(tau_mem)
    tau_s = np.float32(tau_syn)
    tgt = np.float32(target_rate)
    lr = np.float32(homeostatic_lr)
    a_mem = np.float32(np.exp(np.float64(np.float32(-DT) / tau_m)))
    a_syn = np.float32(np.exp(np.float64(np.float32(-DT) / tau_s)))

    wscale = None
    if MATMUL_MODE == "fp8x4":
        wscale = float(np.float32(W_SCALE_NUM / max(np.abs(w_eff).max(),
                                                    1e-30)))
    nc = _get_nc(a_mem, a_syn, lr, tgt, wscale)

    # spikes [B,I,T] -> [k, c, i128, b, tc] contiguous (shared by all cores)
    sIT = spikes.transpose(1, 0, 2)      # [I, B, T]
    pieces = []
    t0 = 0
    for tc_ in CHUNKS:
        blk = sIT[:, :, t0:t0 + tc_].reshape(I, B * tc_)
        pieces.append(blk)
        t0 += tc_
    spk_prep = np.ascontiguousarray(np.concatenate(pieces, axis=1))  # [I, B*T]
    spk8_prep = None
    if MATMUL_MODE in ("bf16x2", "bf16fp8", "fp8x4"):
        import ml_dtypes
        if MATMUL_MODE in ("bf16fp8", "fp8x4"):
            spk8_prep = spk_prep.astype(ml_dtypes.float8_e4m3)
            spk_prep = spk_prep.astype(np.float16)
        else:
            spk_prep = spk_prep.astype(ml_dtypes.bfloat16)

    in_maps = []
    for core in range(NCORES):
        shard = w_eff[:, core * HL:(core + 1) * HL]          # [I, 256]
        wk = shard.reshape(KT, 128, 2, 128).transpose(0, 2, 1, 3)  # [k,ht,i,h]
        wk8 = None
        if MATMUL_MODE == "fp8x4":
            import ml_dtypes
            r = wk * np.float32(wscale)             # [k,ht,i,h] scaled
            planes = []
            for _p in range(NPLANES):
                q = r.astype(ml_dtypes.float8_e4m3)
                r = r - q.astype(np.float32)
                # [k,ht,i,h] -> [kp,ko,ht,i,h] -> [i,kp,ht,ko,h]
                planes.append(
                    q.reshape(KT // 2, 2, 2, 128, 128)
                    .transpose(3, 0, 2, 1, 4).reshape(128, KT * 2 * 128))
            wk8 = np.ascontiguousarray(np.stack(planes, axis=1)).reshape(
                128, NPLANES * KT * 2 * 128)
        elif MATMUL_MODE == "bf16x2":
            import ml_dtypes
            whi = wk.astype(ml_dtypes.bfloat16)
            wlo = (wk - whi.astype(np.float32)).astype(ml_dtypes.bfloat16)
            wk = np.stack([whi, wlo], axis=0)               # [p,k,ht,i,h]
            wk = np.ascontiguousarray(wk.transpose(3, 0, 1, 2, 4)
                                      ).reshape(128, 2 * KT * 2 * 128)
        elif MATMUL_MODE == "bf16fp8":
            import ml_dtypes
            whi = wk.astype(np.float16)                     # [k,ht,i,h]
            wlo = (wk - whi.astype(np.float32)) * np.float32(LO_SCALE)
            # [k,ht,i,h] -> [kp,ko,ht,i,h] -> [i,kp,ht,ko,h]
            wlo = wlo.reshape(KT // 2, 2, 2, 128, 128).transpose(3, 0, 2, 1, 4)
            wk8 = np.ascontiguousarray(wlo.astype(ml_dtypes.float8_e4m3)
                                       ).reshape(128, KT * 2 * 128)
            whiS = (whi.astype(np.float32)
                    * np.float32(LO_SCALE)).astype(np.float16)
            wk = np.ascontiguousarray(whiS.transpose(2, 0, 1, 3)
                                      ).reshape(128, KT * 2 * 128)
        else:
            wk = np.ascontiguousarray(wk.transpose(2, 0, 1, 3)
                                      ).reshape(128, KT * 2 * 128)
        nt0 = np.ascontiguousarray(
            -thr[core * HL:(core + 1) * HL].reshape(2, 128).T)
        im = {"nt0": nt0}
        if MATMUL_MODE == "fp8x4":
            im["wgt8"] = wk8
            im["spk8"] = spk8_prep
        else:
            im["wgt"] = wk
            if not (MATMUL_MODE == "bf16fp8" and CAST_HI):
                im["spk"] = spk_prep
            if MATMUL_MODE == "bf16fp8":
                im["wgt8"] = wk8
                im["spk8"] = spk8_prep
        in_maps.append(im)

    res = run_bass_kernel_spmd(nc, in_maps, core_ids=list(range(NCORES)),
                               trace=TRACE, **TRACE_KW)
    kernel.last_result = res

    outs = []
    for core in range(NCORES):
        o = res.results[core]["out"]
        if MATMUL_MODE == "fp8x4":
            # LIF_SC emits cc-scaled spikes; recover {0,1}
            o = (o.astype(np.float32) != 0.0).astype(np.float32)
        else:
            o = o.astype(np.float32)
        o = o.reshape(128, T, 2, B)
        outs.append(o.transpose(3, 2, 0, 1).reshape(B, HL, T))
    return np.ascontiguousarray(np.concatenate(outs, axis=1))
